# revision 44
# baseline (speedup 1.0000x reference)
"""Trainium2 Bass kernel for nn_Encoder_Decoder_Wrapper (conv encoder -> NTM step -> conv decoder).

Sharding: pure data parallel, batch 64 -> 8 cores x 8 samples. Weights replicated.

Per core, samples are processed in 4 pairs of 2 so every 64-channel conv runs as
K=128/M=128 block-diagonal matmuls (2 samples packed in both contraction and
output partitions).  All conv matmuls use float32r (fp22, 1 cycle/row at N>=256).

The NTM step is algebraically reduced using its constant initial state:
  - reads0 = h0 = c0 = 0  =>  z = x @ w_lstm_x[:256, (i,g,o)] + b  (f gate unused)
  - memory M == 1e-6 everywhere and the post-read writes are discarded, so
    content addressing of the constant memory gives exactly uniform weights;
    the read vectors collapse to reads[b,h,:] = 1e-6 * S(gamma_h) with
    S = q/(q+1e-8), q = 64*(1/64+1e-16)^gamma, gamma = softplus(clip(p)) + 1,
    where p = h @ w_param[:, 262*h+261] + b_param[262*h+261].
  - their contribution to the output is reads_flat @ w_out[256:], i.e.
    sum_h 1e-6*S_h * colsum_h with colsum_h = w_out[256+256h : 512+256h].sum(0).

Performance notes (v2): the PE has a p-state/HAM ramp that resets on idle, so
the kernel keeps the PE streaming continuously: conv0 taps are staged as
contiguous flat-shifted DMA copies (wrap garbage confined to border cells that
are memset), PSUM evictions are batched into 2-bank groups spread across
ACT/DVE/GPSIMD, conv2 runs as a K=2 tap accumulation straight out of the NTM
output tile, and keep-warm matmuls chained off NTM intermediates bridge the
serial NTM window.
"""

import os
import sys

sys.path.insert(0, "/opt/trn_rl_repo")
os.environ.setdefault("MYCRO_LOCAL_CACHE", "1")

import numpy as np

import concourse.bass as bass
import concourse.bacc as bacc
import concourse.mybir as mybir
import concourse.tile as tile
from concourse.masks import make_identity

F32 = mybir.dt.float32
F32R = mybir.dt.float32r
AF = mybir.ActivationFunctionType
ALU = mybir.AluOpType

TAPS = [(dy, dx) for dy in range(3) for dx in range(3)]
LN64 = 4.1588830833596715
CLIP = 20.0

N_CORES = 8
B_CORE = 8          # samples per core
NPAIR = B_CORE // 2
P0 = 64 * 64


def build_nc(debug=False):
    nc = bacc.Bacc(None, target_bir_lowering=False)

    inp = nc.dram_tensor("inputs", [B_CORE, 1, 64, 64], F32R, kind="ExternalInput")
    wc0 = nc.dram_tensor("w_conv0", [64, 1, 3, 3], F32, kind="ExternalInput")
    bc0 = nc.dram_tensor("b_conv0", [64], F32, kind="ExternalInput")
    wc1 = nc.dram_tensor("w_conv1", [64, 64, 3, 3], F32, kind="ExternalInput")
    bc1 = nc.dram_tensor("b_conv1", [64], F32, kind="ExternalInput")
    wen = nc.dram_tensor("w_enc", [1, 64, 3, 3], F32, kind="ExternalInput")
    ben = nc.dram_tensor("b_enc", [1], F32, kind="ExternalInput")
    wc2 = nc.dram_tensor("w_conv2", [64, 1, 3, 3], F32, kind="ExternalInput")
    bc2 = nc.dram_tensor("b_conv2", [64], F32, kind="ExternalInput")
    wc3 = nc.dram_tensor("w_conv3", [64, 64, 3, 3], F32, kind="ExternalInput")
    bc3 = nc.dram_tensor("b_conv3", [64], F32, kind="ExternalInput")
    wc4 = nc.dram_tensor("w_conv4", [64, 64, 3, 3], F32, kind="ExternalInput")
    bc4 = nc.dram_tensor("b_conv4", [64], F32, kind="ExternalInput")
    wlx = nc.dram_tensor("w_lstm_x", [1024, 1024], F32R, kind="ExternalInput")
    bls = nc.dram_tensor("b_lstm", [1024], F32, kind="ExternalInput")
    wpa = nc.dram_tensor("w_param", [256, 3108], F32R, kind="ExternalInput")
    bpa = nc.dram_tensor("b_param", [3108], F32, kind="ExternalInput")
    wou = nc.dram_tensor("w_out", [1024, 256], F32R, kind="ExternalInput")
    bou = nc.dram_tensor("b_out", [256], F32R, kind="ExternalInput")
    out = nc.dram_tensor("out", [B_CORE, 64, 64, 64], F32, kind="ExternalOutput")
    # zero-padded 66-pitch staging copy of the input; ExternalOutput =>
    # pre-zeroed by the runtime each invocation, so pad cells are true zeros
    zscr = nc.dram_tensor("zscr", [B_CORE, 4368], F32R, kind="ExternalOutput")

    dbg = {}
    if debug:
        for name, shape in [
            ("dbg_pat", [18, 64, 66]),
            ("dbg_c1in", [128, 34, 34]),
            ("dbg_ein", [128, 18, 18]),
            ("dbg_x", [8, 16, 16]),
            ("dbg_stg", [2, 4, 18, 18]),
            ("dbg_c3in", [128, 17, 2, 17, 2]),
            ("dbg_c4in", [128, 33, 2, 33, 2]),
        ]:
            dbg[name] = nc.dram_tensor(name, shape, F32R, kind="ExternalOutput")

    with tile.TileContext(nc) as tc:
        with (
            tc.tile_pool(name="const", bufs=1) as const,
            tc.tile_pool(name="work", bufs=1) as work,
            tc.tile_pool(name="c1p", bufs=4) as c1p,
            tc.tile_pool(name="einp", bufs=4) as einp,
            tc.tile_pool(name="evp", bufs=4) as evp,
            tc.tile_pool(name="p1p", bufs=3) as p1p,
            tc.tile_pool(name="c3p", bufs=3) as c3p,
            tc.tile_pool(name="c4p", bufs=2) as c4p,
            tc.tile_pool(name="psg", bufs=3, space="PSUM") as psg,
        ):
            # ---------------- identity + relu act-table warm ----------------
            ident = const.tile([128, 128], F32, tag="ident")
            make_identity(nc, ident)
            onesc = const.tile([128, 1], F32R, tag="onesc")
            nc.vector.memset(onesc[:].bitcast(F32), 1.0)
            tblw = const.tile([128, 1], F32, tag="tblw")
            nc.vector.memset(tblw[:], 0.0)
            nc.scalar.activation(tblw[:], tblw[:], AF.Relu, bias=0.0)

            # early weight loads needed for conv0/conv1 builds
            s9 = const.tile([64, 9], F32, tag="w9_c0")
            nc.gpsimd.dma_start(out=s9[:], in_=wc0[:].rearrange("a b c d -> a (b c d)"))
            wsrc1 = const.tile([64, 576], F32, tag="wsrc1")
            nc.gpsimd.dma_start(
                out=wsrc1[:], in_=wc1[:].rearrange("a b c d -> a (b c d)")
            )
            def bias128(dram_b, tag, eng):
                bt = const.tile([128, 1], F32, tag=tag)
                eng.dma_start(
                    out=bt[:],
                    in_=bass.AP(tensor=dram_b[:].tensor, offset=0,
                                ap=[[0, 2], [1, 64], [0, 1]]),
                )
                return bt

            # ---------------- conv0 input staging ---------------------------
            # One padded-copy DMA of the whole input into the pre-zeroed
            # 66-pitch DRAM scratch, then per (pair, sample, dy) tap DMAs:
            # taps t=3*dy+dx of sample s01 sit at partitions 16*s01 + t of a
            # per-pair tile, so pair 0 only waits on its own 6 tap DMAs.
            PP = 64 * 66
            PPb = 64 * 66
            dma_engs = [nc.sync, nc.scalar, nc.gpsimd]
            dma_rr = [0]

            def nxt_eng():
                e = dma_engs[dma_rr[0] % 3]
                dma_rr[0] += 1
                return e

            nc.sync.dma_start(
                out=bass.AP(tensor=zscr[:].tensor, offset=67,
                            ap=[[4368, 8], [66, 64], [1, 64]]),
                in_=bass.AP(tensor=inp[:].tensor, offset=0,
                            ap=[[4096, 8], [66, 64] if False else [64, 64], [1, 64]]),
            )
            HH = 32 * 66
            pat_t = const.tile([128, 32, 66], F32R, tag="pat_t")
            pat_b = const.tile([128, 32, 66], F32R, tag="pat_b")
            for half, tile_h in ((0, pat_t), (1, pat_b)):
                base = tile_h[:].offset
                tpat = tile_h[:].tensor
                for t, (dy, dx) in enumerate(TAPS):
                    nxt_eng().dma_start(
                        out=bass.AP(
                            tensor=tpat,
                            offset=base + t * HH,
                            ap=[[16 * HH, 8], [1, HH]],
                        ),
                        in_=bass.AP(
                            tensor=zscr[:].tensor,
                            offset=dy * 66 + dx + half * HH,
                            ap=[[4368, 8], [1, HH]],
                        ),
                    )
            if debug:
                nc.sync.dma_start(out=dbg["dbg_pat"][0:9, 0:32, :], in_=pat_t[0:9])
                nc.sync.dma_start(out=dbg["dbg_pat"][0:9, 32:64, :], in_=pat_b[0:9])
                nc.sync.dma_start(out=dbg["dbg_pat"][9:18, 0:32, :], in_=pat_t[16:25])
                nc.sync.dma_start(out=dbg["dbg_pat"][9:18, 32:64, :], in_=pat_b[16:25])
            bt0 = bias128(bc0, "bt0", nc.scalar)

            # ---------------- conv0 weights -> [18,128] lhsT ----------------
            ct0w = const.tile([128, 128], F32R, tag="cT_c0")
            nc.vector.memset(ct0w[:].bitcast(F32), 0.0)
            p9 = psg.tile([9, 64], F32, tag="g2")
            nc.tensor.transpose(p9[:], s9[:], ident[0:64, 0:64])
            nc.vector.tensor_scalar_mul(ct0w[0:9, 0:64], p9[:], 1.0)
            nc.sync.dma_start(out=ct0w[16:25, 64:128], in_=ct0w[0:9, 0:64])
            for p, eng in ((1, nc.scalar), (2, nc.gpsimd), (3, nc.sync)):
                eng.dma_start(
                    out=ct0w[32 * p : 32 * p + 25, :], in_=ct0w[0:25, :]
                )
            wsrc3 = const.tile([64, 576], F32, tag="wsrc3")
            nc.sync.dma_start(
                out=wsrc3[:], in_=wc3[:].rearrange("a b c d -> a (b c d)")
            )
            wsrc4 = const.tile([64, 576], F32, tag="wsrc4")
            nc.sync.dma_start(
                out=wsrc4[:], in_=wc4[:].rearrange("a b c d -> a (b c d)")
            )

            # ================ conv0 + pool (4 groups of 2 banks per pair) ===
            # top-half n-tiles of every pair first: they only need pat_t, so
            # the PE starts while the bottom-half taps are still staging.
            c1in_l = []
            for p in range(NPAIR):
                c1in = c1p.tile([128, 34, 34], F32R, tag="c1in")
                nc.vector.memset(c1in[:, 0:1, :].bitcast(F32), 0.0)
                nc.vector.memset(c1in[:, 33:34, :].bitcast(F32), 0.0)
                nc.vector.memset(c1in[:, 1:33, 0:1].bitcast(F32), 0.0)
                nc.vector.memset(c1in[:, 1:33, 33:34].bitcast(F32), 0.0)
                c1in_l.append(c1in)

            def conv0_group(p, g):
                c1in = c1in_l[p]
                ps = psg.tile([128, 2, 4, 2, 32, 2], F32, tag="g2")
                for n2 in range(2):
                    n = 2 * g + n2
                    src_t = pat_t if n < 4 else pat_b
                    nn = n if n < 4 else n - 4
                    nc.tensor.matmul(
                        ps[:, n2],
                        ct0w[32 * p : 32 * p + 25, :],
                        src_t[32 * p : 32 * p + 25, nn * 8 : nn * 8 + 8, 0:64],
                        start=True,
                        stop=True,
                        tile_position=(32 * p, 0),
                    )
                R = evp.tile([128, 2, 4, 2, 32, 2], F32, tag="evict")
                if g % 2 == 0:
                    nc.scalar.activation(R[:], ps[:], AF.Relu, bias=bt0)
                else:
                    nc.vector.tensor_scalar(
                        R[:], ps[:], bt0[:], 0.0, ALU.add, ALU.max
                    )
                T1 = p1p.tile([128, 2, 4, 2, 32], F32, tag="pool1")
                nc.vector.tensor_add(
                    T1[:], R[:, :, :, :, :, 0], R[:, :, :, :, :, 1]
                )
                nc.gpsimd.tensor_add(
                    c1in[:, 1 + 8 * g : 9 + 8 * g, 1:33],
                    T1[:, :, :, 0, :].rearrange("p a b c -> p (a b) c"),
                    T1[:, :, :, 1, :].rearrange("p a b c -> p (a b) c"),
                )

            for p in range(NPAIR):
                conv0_group(p, 0)
                conv0_group(p, 1)
            for p in range(NPAIR):
                conv0_group(p, 2)
                conv0_group(p, 3)
            wencs = const.tile([64, 9], F32, tag="wencs")
            nc.sync.dma_start(
                out=wencs[:], in_=wen[:].rearrange("a b c d -> (a b) (c d)")
            )
            encT = const.tile([128, 9, 2], F32R, tag="encT")
            nc.vector.memset(encT[:].bitcast(F32), 0.0)
            for t in range(9):
                nc.scalar.activation(
                    encT[0:64, t, 0:1], wencs[:, t : t + 1], AF.Copy,
                    bias=0.0, scale=0.25,
                )
            nc.gpsimd.dma_start(out=encT[64:128, :, 1:2], in_=encT[0:64, :, 0:1])
            bte = const.tile([2, 1], F32, tag="bte")
            nc.sync.dma_start(
                out=bte[:],
                in_=bass.AP(tensor=ben[:].tensor, offset=0, ap=[[0, 2], [1, 1]]),
            )

            # ---------------- conv1 weights (block-diag taps) ---------------
            def build_wtap(tag, wsrc_tile, scale):
                wt = const.tile([128, 9, 128], F32R, tag=tag)
                nc.vector.memset(wt[:].bitcast(F32), 0.0)
                for t in range(9):
                    pw = psg.tile([64, 64], F32, tag="g2")
                    nc.tensor.transpose(pw[:], wsrc_tile[:, t::9], ident[0:64, 0:64])
                    if scale == 1.0:
                        nc.vector.tensor_scalar_mul(wt[0:64, t, 0:64], pw[:], 1.0)
                    else:
                        nc.scalar.activation(
                            wt[0:64, t, 0:64], pw[:], AF.Copy, bias=0.0, scale=scale
                        )
                nc.sync.dma_start(out=wt[64:128, :, 64:128], in_=wt[0:64, :, 0:64])
                return wt

            wtap1 = build_wtap("wtap_c1", wsrc1, 0.25)  # avg-pool folded in
            bt1 = bias128(bc1, "bt1", nc.sync)

            # ================ conv1 + pool (one 2-bank group per pair) ======
            ein_l = []
            for p in range(NPAIR):
                c1in = c1in_l[p]
                e_in = einp.tile([128, 18, 18], F32R, tag="e_in")
                nc.vector.memset(e_in[:, 0:1, :].bitcast(F32), 0.0)
                nc.vector.memset(e_in[:, 17:18, :].bitcast(F32), 0.0)
                nc.vector.memset(e_in[:, 1:17, 0:1].bitcast(F32), 0.0)
                nc.vector.memset(e_in[:, 1:17, 17:18].bitcast(F32), 0.0)
                ps = psg.tile([128, 2, 8, 2, 16, 2], F32, tag="g2")
                for n in range(2):
                    for t, (dy, dx) in enumerate(TAPS):
                        nc.tensor.matmul(
                            ps[:, n],
                            wtap1[:, t, :],
                            c1in[:, n * 16 + dy : n * 16 + dy + 16, dx : dx + 32],
                            start=(t == 0),
                            stop=(t == 8),
                        )
                R = evp.tile([128, 2, 8, 2, 16, 2], F32, tag="evict")
                if p % 2 == 0:
                    nc.scalar.activation(R[:], ps[:], AF.Relu, bias=bt1)
                else:
                    nc.vector.tensor_scalar(R[:], ps[:], bt1[:], 0.0, ALU.add, ALU.max)
                T1 = p1p.tile([128, 2, 8, 2, 16], F32, tag="pool1")
                nc.vector.tensor_add(T1[:], R[:, :, :, :, :, 0], R[:, :, :, :, :, 1])
                nc.gpsimd.tensor_add(
                    e_in[:, 1:17, 1:17],
                    T1[:, :, :, 0, :].rearrange("p a b c -> p (a b) c"),
                    T1[:, :, :, 1, :].rearrange("p a b c -> p (a b) c"),
                )
                if debug and p == 0:
                    nc.sync.dma_start(out=dbg["dbg_ein"][:], in_=e_in[:])
                ein_l.append(e_in)

            # sigmoid/tanh table load overlaps late encoder
            nc.scalar.activation(tblw[:], tblw[:], AF.Sigmoid, bias=0.0)

            # ---------------- deferred weights: enc, conv2, conv3, NTM ------

            s9c2 = const.tile([64, 9], F32R, tag="w9_c2")
            nc.gpsimd.dma_start(
                out=s9c2[:], in_=wc2[:].rearrange("a b c d -> a (b c d)")
            )
            c2T = const.tile([2, 128, 9], F32R, tag="c2T")
            nc.vector.memset(c2T[:].bitcast(F32), 0.0)
            for r in range(2):
                nc.gpsimd.dma_start(
                    out=bass.AP(
                        tensor=c2T[:].tensor,
                        offset=c2T[:].offset + r * (128 * 9) + r * 576,
                        ap=[[128 * 9, 1], [1, 576]],
                    ),
                    in_=bass.AP(
                        tensor=s9c2[:].tensor, offset=s9c2[:].offset,
                        ap=[[9, 64], [1, 9]],
                    ),
                )
            stg2 = const.tile([2, 4, 18, 18], F32R, tag="stg2")
            nc.vector.memset(stg2[:].bitcast(F32), 0.0)

            wx = const.tile([128, 2, 768], F32R, tag="wx")
            for kt in range(2):
                nc.sync.dma_start(
                    out=wx[:, kt, 0:256],
                    in_=wlx[kt * 128 : (kt + 1) * 128, 0:256],
                )
                nc.sync.dma_start(
                    out=wx[:, kt, 256:768],
                    in_=wlx[kt * 128 : (kt + 1) * 128, 512:1024],
                )
            bigo = const.tile([128, 6], F32, tag="bigo")
            nc.sync.dma_start(
                out=bigo[:, 0:2],
                in_=bass.AP(tensor=bls[:].tensor, offset=0,
                            ap=[[1, 128], [128, 2]]),
            )
            nc.sync.dma_start(
                out=bigo[:, 2:6],
                in_=bass.AP(tensor=bls[:].tensor, offset=512,
                            ap=[[1, 128], [128, 4]]),
            )
            wp3 = const.tile([128, 2, 3], F32R, tag="wp3")
            for kt in range(2):
                nc.sync.dma_start(
                    out=wp3[:, kt, :],
                    in_=bass.AP(
                        tensor=wpa[:].tensor,
                        offset=kt * 128 * 3108 + 261,
                        ap=[[3108, 128], [262, 3]],
                    ),
                )
            bp3 = const.tile([3, 1], F32, tag="bp3")
            nc.sync.dma_start(
                out=bp3[:],
                in_=bass.AP(tensor=bpa[:].tensor, offset=261, ap=[[262, 3], [1, 1]]),
            )
            wo = const.tile([128, 2, 256], F32R, tag="wo")
            for kt in range(2):
                nc.sync.dma_start(
                    out=wo[:, kt, :], in_=wou[kt * 128 : (kt + 1) * 128, :]
                )
            w2c = const.tile([128, 6, 256], F32R, tag="w2c")
            nc.gpsimd.dma_start(
                out=w2c[:],
                in_=bass.AP(tensor=wou[:].tensor, offset=256 * 256,
                            ap=[[256, 128], [128 * 256, 6], [1, 256]]),
            )
            ones3 = const.tile([128, 6, 3], F32R, tag="ones3")
            nc.vector.memset(ones3[:].bitcast(F32), 0.0)
            for c in range(6):
                nc.vector.memset(ones3[:, c, c // 2 : c // 2 + 1].bitcast(F32), 1.0)
            rhs2 = const.tile([4, 256], F32R, tag="rhs2")
            nc.sync.dma_start(out=rhs2[3:4, :], in_=bou[:].unsqueeze(0))
            lhsT2 = const.tile([4, 8], F32R, tag="lhsT2")
            nc.vector.memset(lhsT2[:].bitcast(F32), 1.0)
            bt2 = bias128(bc2, "bt2", nc.sync)
            bt3 = bias128(bc3, "bt3", nc.sync)
            bt4 = bias128(bc4, "bt4", nc.sync)

            wtap3 = build_wtap("wtap_c3", wsrc3, 1.0)

            # ================ enc conv (M=2 per pair, DMA into xstage) ======
            xstage = const.tile([8, 16, 16], F32, tag="xstage")
            scrw = psg.tile([1, 512], F32, tag="ntm", bufs=1, name="scrw")
            wdst = scrw[:]

            def warm(dep, n=1):
                ap = dep
                free = 1
                for dd in ap.ap[1:]:
                    free *= dd[1]
                reps = 512 // free
                k = ap.ap[0][1]
                rhs = bass.AP(
                    tensor=ap.tensor, offset=ap.offset,
                    ap=[list(ap.ap[0])] + [[0, reps]] + [list(dd) for dd in ap.ap[1:]],
                )
                for _ in range(n):
                    nc.tensor.matmul(wdst, onesc[0:k, 0:1], rhs, start=True, stop=True)

            estage_l = []
            for p in range(NPAIR):
                e_in = ein_l[p]
                pe = psg.tile([2, 16, 16], F32, tag="g2")
                for t, (dy, dx) in enumerate(TAPS):
                    nc.tensor.matmul(
                        pe[:],
                        encT[:, t, :],
                        e_in[:, dy : dy + 16, dx : dx + 16],
                        start=(t == 0),
                        stop=(t == 8),
                    )
                estage = work.tile([2, 16, 16], F32R, tag=f"estage{p}")
                nc.scalar.activation(estage[:], pe[:], AF.Relu, bias=bte)
                nc.gpsimd.dma_start(out=xstage[2 * p : 2 * p + 2, :, :], in_=estage[:])
                warm(estage[:], n=2)
                estage_l.append(estage)

            if debug:
                nc.gpsimd.dma_start(out=dbg["dbg_x"][:], in_=xstage[:])
            # ================ NTM step ======================================
            xT = work.tile([128, 2, 8], F32R, tag="xT")
            for kt in range(2):
                pxt = psg.tile([128, 8], F32, tag="g2")
                nc.tensor.transpose(
                    pxt[:],
                    xstage[:].rearrange("p a b -> p (a b)")[:, kt * 128 : kt * 128 + 128],
                    ident[0:8, 0:8],
                )
                nc.scalar.activation(xT[:, kt, :], pxt[:], AF.Copy, bias=0.0, scale=1.0)

            # w_out reads-part colsums -> rhs2 rows 0:3
            pcs = psg.tile([3, 256], F32, tag="g2")
            for c in range(6):
                nc.tensor.matmul(
                    pcs[:], ones3[:, c, :], w2c[:, c, :],
                    start=(c == 0), stop=(c == 5),
                )
            nc.scalar.activation(rhs2[0:3, :], pcs[:], AF.Copy, bias=0.0, scale=1.0)

            # z = x @ Wx + b for gates i, g, o
            zps = psg.tile([128, 6, 8], F32, tag="g2")
            for j in range(3):
                for h2 in range(2):
                    for kt in range(2):
                        nc.tensor.matmul(
                            zps[:, 2 * j + h2, :],
                            wx[:, kt, j * 256 + h2 * 128 : j * 256 + h2 * 128 + 128],
                            xT[:, kt, :],
                            start=(kt == 0),
                            stop=(kt == 1),
                        )
            zb = work.tile([128, 6, 8], F32, tag="zb")
            bigo_b = bass.AP(
                tensor=bigo[:].tensor, offset=bigo[:].offset,
                ap=[list(d) for d in bigo[:].ap] + [[0, 8]],
            )
            nc.vector.tensor_tensor(zb[:], zps[:], bigo_b, op=ALU.add)

            si = work.tile([128, 2, 8], F32R, tag="gate0")
            nc.scalar.activation(si[:], zb[:, 0:2, :], AF.Sigmoid, bias=0.0)
            tg = work.tile([128, 2, 8], F32R, tag="gate1")
            nc.scalar.activation(tg[:], zb[:, 2:4, :], AF.Tanh, bias=0.0)
            so = work.tile([128, 2, 8], F32R, tag="gate2")
            nc.scalar.activation(so[:], zb[:, 4:6, :], AF.Sigmoid, bias=0.0)
            warm(si[:], n=3)
            ctile = work.tile([128, 2, 8], F32R, tag="ctile")
            nc.vector.tensor_mul(ctile[:], si[:], tg[:])
            warm(so[:], n=3)
            tct = work.tile([128, 2, 8], F32R, tag="tct")
            nc.scalar.activation(tct[:], ctile[:], AF.Tanh, bias=0.0)
            warm(ctile[:], n=3)
            h = work.tile([128, 2, 8], F32R, tag="h")
            nc.vector.tensor_mul(h[:], so[:], tct[:])
            warm(tct[:], n=3)

            # conv4 taps built inside the NTM window (PE work + DVE evicts)
            wt4 = const.tile([128, 9, 128], F32R, tag="wtap_c4")
            nc.gpsimd.memset(wt4[:].bitcast(F32), 0.0)

            pp3 = psg.tile([3, 8], F32, tag="g2")
            for kt in range(2):
                nc.tensor.matmul(
                    pp3[:], wp3[:, kt, :], h[:, kt, :], start=(kt == 0), stop=(kt == 1)
                )
            t1 = work.tile([3, 8], F32R, tag="t1")
            nc.scalar.activation(t1[:], pp3[:], AF.Identity, bias=bp3)
            t2 = work.tile([3, 8], F32R, tag="t2")
            nc.vector.tensor_scalar(t2[:], t1[:], -CLIP, CLIP, ALU.max, ALU.min)
            for t in range(4):
                pw = psg.tile([64, 64], F32, tag="g2")
                nc.tensor.transpose(pw[:], wsrc4[:, t::9], ident[0:64, 0:64])
                nc.vector.tensor_scalar_mul(wt4[0:64, t, 0:64], pw[:], 1.0)
            eu = work.tile([3, 8], F32R, tag="eu")
            nc.scalar.activation(eu[:], t2[:], AF.Exp, bias=0.0)
            ev = work.tile([3, 8], F32R, tag="ev")
            nc.vector.tensor_scalar_add(ev[:], eu[:], 1.0)
            warm(t2[:], n=3)
            sp = work.tile([3, 8], F32R, tag="sp")
            nc.scalar.activation(sp[:], ev[:], AF.Ln, bias=0.0)
            for t in range(4, 9):
                pw = psg.tile([64, 64], F32, tag="g2")
                nc.tensor.transpose(pw[:], wsrc4[:, t::9], ident[0:64, 0:64])
                nc.vector.tensor_scalar_mul(wt4[0:64, t, 0:64], pw[:], 1.0)
            q = work.tile([3, 8], F32R, tag="q")
            nc.scalar.activation(q[:], sp[:], AF.Exp, bias=0.0, scale=-LN64)
            qe = work.tile([3, 8], F32R, tag="qe")
            nc.vector.tensor_scalar_add(qe[:], q[:], 1e-8)
            warm(sp[:], n=3)
            rec = work.tile([3, 8], F32, tag="rec")
            nc.vector.reciprocal(rec[:], qe[:])
            nc.vector.scalar_tensor_tensor(
                out=lhsT2[0:3, :], in0=q[:], scalar=1e-6, in1=rec[:],
                op0=ALU.mult, op1=ALU.mult,
            )
            nc.gpsimd.dma_start(out=wt4[64:128, :, 64:128], in_=wt4[0:64, :, 0:64])
            warm(qe[:], n=3)

            # out = clip(h @ w_out[:256] + reads @ w_out[256:] + b_out),
            # per pair (M=2) so the clip lands lane-aligned in stg2
            pout = psg.tile([2, 4, 16, 16], F32, tag="g2")
            for p in range(NPAIR):
                for kt in range(2):
                    nc.tensor.matmul(
                        pout[:, p],
                        h[:, kt, 2 * p : 2 * p + 2],
                        wo[:, kt, :],
                        start=(kt == 0),
                        stop=False,
                    )
                nc.tensor.matmul(
                    pout[:, p],
                    lhsT2[:, 2 * p : 2 * p + 2],
                    rhs2[:],
                    start=False,
                    stop=True,
                )
                nc.vector.tensor_scalar(
                    stg2[:, p, 1:17, 1:17], pout[:, p],
                    -CLIP, CLIP, ALU.max, ALU.min,
                )

            if debug:
                nc.sync.dma_start(out=dbg["dbg_stg"][:], in_=stg2[:])
            # ================ decoder ======================================
            c3in_l = []
            ps2_l = []

            def conv2_pair(p):
                ps2 = psg.tile([128, 16, 16], F32, tag="g2")
                for t, (dy, dx) in enumerate(TAPS):
                    nc.tensor.matmul(
                        ps2[:],
                        c2T[:, :, t],
                        stg2[:, p, dy : dy + 16, dx : dx + 16],
                        start=(t == 0),
                        stop=(t == 8),
                    )
                ps2_l.append(ps2)

            def c3asm_pair(p):
                psv = ps2_l[p][:]
                c3in = c3p.tile([128, 17, 2, 17, 2], F32R, tag="c3in")
                nc.gpsimd.memset(c3in[:, 0, 0, :, :].bitcast(F32), 0.0)
                nc.gpsimd.memset(c3in[:, 16, 1, :, :].bitcast(F32), 0.0)
                nc.gpsimd.memset(c3in[:, :, :, 0, 0].bitcast(F32), 0.0)
                nc.gpsimd.memset(c3in[:, :, :, 16, 1].bitcast(F32), 0.0)
                nc.scalar.activation(c3in[:, 0:16, 1, 0:16, 1], psv, AF.Relu, bias=bt2)
                nc.vector.tensor_scalar(
                    c3in[:, 0:16, 1, 1:17, 0], psv, bt2[:], 0.0, ALU.add, ALU.max
                )
                nc.scalar.activation(c3in[:, 1:17, 0, 0:16, 1], psv, AF.Relu, bias=bt2)
                nc.vector.tensor_scalar(
                    c3in[:, 1:17, 0, 1:17, 0], psv, bt2[:], 0.0, ALU.add, ALU.max
                )
                if debug and p == 0:
                    nc.sync.dma_start(out=dbg["dbg_c3in"][:], in_=c3in[:])
                c3in_l.append(c3in)

            c4in_l = []

            def conv3_pair(p):
                c3v = c3in_l[p][:].rearrange("p r a c b -> p (r a) (c b)")
                ps = psg.tile([128, 2, 16, 32], F32, tag="g2")
                for n in range(2):
                    for t, (dy, dx) in enumerate(TAPS):
                        nc.tensor.matmul(
                            ps[:, n],
                            wtap3[:, t, :],
                            c3v[:, n * 16 + dy : n * 16 + dy + 16, dx : dx + 32],
                            start=(t == 0),
                            stop=(t == 8),
                        )
                psv = ps[:].rearrange("p n a c -> p (n a) c")
                c4in = c4p.tile([128, 33, 2, 33, 2], F32R, tag="c4in")
                nc.gpsimd.memset(c4in[:, 0, 0, :, :].bitcast(F32), 0.0)
                nc.gpsimd.memset(c4in[:, 32, 1, :, :].bitcast(F32), 0.0)
                nc.gpsimd.memset(c4in[:, :, :, 0, 0].bitcast(F32), 0.0)
                nc.gpsimd.memset(c4in[:, :, :, 32, 1].bitcast(F32), 0.0)
                nc.scalar.activation(c4in[:, 0:32, 1, 0:32, 1], psv, AF.Relu, bias=bt3)
                nc.vector.tensor_scalar(
                    c4in[:, 0:32, 1, 1:33, 0], psv, bt3[:], 0.0, ALU.add, ALU.max
                )
                nc.scalar.activation(c4in[:, 1:33, 0, 0:32, 1], psv, AF.Relu, bias=bt3)
                nc.vector.tensor_scalar(
                    c4in[:, 1:33, 0, 1:33, 0], psv, bt3[:], 0.0, ALU.add, ALU.max
                )
                if debug and p == 0:
                    nc.sync.dma_start(out=dbg["dbg_c4in"][:], in_=c4in[:])
                c4in_l.append(c4in)

            def conv4_pair(p):
                c4v = c4in_l[p][:].rearrange("p r a c b -> p (r a) (c b)")
                for g in range(4):
                    ps = psg.tile([128, 2, 8, 64], F32, tag="g2")
                    for t, (dy, dx) in enumerate(TAPS):
                        for n2 in range(2):
                            n = 2 * g + n2
                            nc.tensor.matmul(
                                ps[:, n2],
                                wt4[:, t, :],
                                c4v[:, n * 8 + dy : n * 8 + dy + 8, dx : dx + 64],
                                start=(t == 0),
                                stop=(t == 8),
                            )
                    R = evp.tile([128, 2, 8, 64], F32, tag="evict")
                    if g % 2 == 0:
                        nc.scalar.activation(R[:], ps[:], AF.Relu, bias=bt4)
                    else:
                        nc.vector.tensor_scalar(
                            R[:], ps[:], bt4[:], 0.0, ALU.add, ALU.max
                        )
                    for s01 in range(2):
                        eng = (nc.sync, nc.gpsimd)[s01]
                        eng.dma_start(
                            out=out[2 * p + s01, :, 16 * g : 16 * g + 16, :],
                            in_=R[64 * s01 : 64 * s01 + 64].rearrange(
                                "p n a c -> p (n a) c"
                            ),
                        )

            conv2_pair(0)
            c3asm_pair(0)
            conv2_pair(1)
            c3asm_pair(1)
            conv2_pair(2)
            c3asm_pair(2)
            conv3_pair(0)
            conv2_pair(3)
            c3asm_pair(3)
            conv3_pair(1)
            conv4_pair(0)
            conv3_pair(2)
            conv4_pair(1)
            conv3_pair(3)
            conv4_pair(2)
            conv4_pair(3)

    nc.compile()
    return nc


_NC_CACHE = {}
LAST_RESULT = None

WEIGHT_NAMES = [
    "w_conv0", "b_conv0", "w_conv1", "b_conv1", "w_enc", "b_enc",
    "w_conv2", "b_conv2", "w_conv3", "b_conv3", "w_conv4", "b_conv4",
    "w_lstm_x", "b_lstm", "w_param", "b_param", "w_out", "b_out",
]


def kernel(**inputs):
    global LAST_RESULT
    from concourse.bass_utils import run_bass_kernel_spmd

    debug = bool(int(os.environ.get("KDEBUG", "0")))
    key = ("nc", debug)
    if key not in _NC_CACHE:
        _NC_CACHE[key] = build_nc(debug=debug)
    nc = _NC_CACHE[key]

    xs = np.ascontiguousarray(np.asarray(inputs["inputs"], dtype=np.float32))
    weights = {
        k: np.ascontiguousarray(np.asarray(inputs[k], dtype=np.float32))
        for k in WEIGHT_NAMES
    }
    in_maps = []
    for c in range(N_CORES):
        m = dict(weights)
        m["inputs"] = xs[c * B_CORE : (c + 1) * B_CORE]
        in_maps.append(m)

    res = run_bass_kernel_spmd(nc, in_maps, core_ids=list(range(N_CORES)))
    LAST_RESULT = res
    return np.concatenate([r["out"] for r in res.results], axis=0)


if __name__ == "__main__":
    nc = build_nc()
    print("built ok")


# revision 45
# speedup vs baseline: 1.1638x; 1.1638x over previous
"""Trainium2 Bass kernel for nn_Encoder_Decoder_Wrapper (conv encoder -> NTM step -> conv decoder).

Sharding: pure data parallel, batch 64 -> 8 cores x 8 samples. Weights replicated.

Per core, samples are processed in 4 pairs of 2 so every 64-channel conv runs as
K=128/M=128 block-diagonal matmuls (2 samples packed in both contraction and
output partitions).  All conv matmuls use float32r (fp22, 1 cycle/row at N>=256).

The NTM step is algebraically reduced using its constant initial state:
  - reads0 = h0 = c0 = 0  =>  z = x @ w_lstm_x[:256, (i,g,o)] + b  (f gate unused)
  - memory M == 1e-6 everywhere and the post-read writes are discarded, so
    content addressing of the constant memory gives exactly uniform weights;
    the read vectors collapse to reads[b,h,:] = 1e-6 * S(gamma_h) with
    S = q/(q+1e-8), q = 64*(1/64+1e-16)^gamma, gamma = softplus(clip(p)) + 1,
    where p = h @ w_param[:, 262*h+261] + b_param[262*h+261].
  - their contribution to the output is reads_flat @ w_out[256:], i.e.
    sum_h 1e-6*S_h * colsum_h with colsum_h = w_out[256+256h : 512+256h].sum(0).

Performance notes (v2): the PE has a p-state/HAM ramp that resets on idle, so
the kernel keeps the PE streaming continuously: conv0 taps are staged as
contiguous flat-shifted DMA copies (wrap garbage confined to border cells that
are memset), PSUM evictions are batched into 2-bank groups spread across
ACT/DVE/GPSIMD, conv2 runs as a K=2 tap accumulation straight out of the NTM
output tile, and keep-warm matmuls chained off NTM intermediates bridge the
serial NTM window.
"""

import os
import sys

sys.path.insert(0, "/opt/trn_rl_repo")
os.environ.setdefault("MYCRO_LOCAL_CACHE", "1")

import numpy as np

import concourse.bass as bass
import concourse.bacc as bacc
import concourse.mybir as mybir
import concourse.tile as tile
from concourse.masks import make_identity

F32 = mybir.dt.float32
F32R = mybir.dt.float32r
AF = mybir.ActivationFunctionType
ALU = mybir.AluOpType

TAPS = [(dy, dx) for dy in range(3) for dx in range(3)]
LN64 = 4.1588830833596715
CLIP = 20.0

N_CORES = 8
B_CORE = 8          # samples per core
NPAIR = B_CORE // 2
P0 = 64 * 64


def build_nc(debug=False):
    nc = bacc.Bacc(None, target_bir_lowering=False)

    inp = nc.dram_tensor("inputs", [B_CORE, 1, 64, 64], F32R, kind="ExternalInput")
    wc0 = nc.dram_tensor("w_conv0", [64, 1, 3, 3], F32, kind="ExternalInput")
    bc0 = nc.dram_tensor("b_conv0", [64], F32, kind="ExternalInput")
    wc1 = nc.dram_tensor("w_conv1", [64, 64, 3, 3], F32, kind="ExternalInput")
    bc1 = nc.dram_tensor("b_conv1", [64], F32, kind="ExternalInput")
    wen = nc.dram_tensor("w_enc", [1, 64, 3, 3], F32, kind="ExternalInput")
    ben = nc.dram_tensor("b_enc", [1], F32, kind="ExternalInput")
    wc2 = nc.dram_tensor("w_conv2", [64, 1, 3, 3], F32, kind="ExternalInput")
    bc2 = nc.dram_tensor("b_conv2", [64], F32, kind="ExternalInput")
    wc3 = nc.dram_tensor("w_conv3", [64, 64, 3, 3], F32, kind="ExternalInput")
    bc3 = nc.dram_tensor("b_conv3", [64], F32, kind="ExternalInput")
    wc4 = nc.dram_tensor("w_conv4", [64, 64, 3, 3], F32, kind="ExternalInput")
    bc4 = nc.dram_tensor("b_conv4", [64], F32, kind="ExternalInput")
    wlx = nc.dram_tensor("w_lstm_x", [1024, 1024], F32R, kind="ExternalInput")
    bls = nc.dram_tensor("b_lstm", [1024], F32, kind="ExternalInput")
    wpa = nc.dram_tensor("w_param", [256, 3108], F32R, kind="ExternalInput")
    bpa = nc.dram_tensor("b_param", [3108], F32, kind="ExternalInput")
    wou = nc.dram_tensor("w_out", [1024, 256], F32R, kind="ExternalInput")
    bou = nc.dram_tensor("b_out", [256], F32R, kind="ExternalInput")
    out = nc.dram_tensor("out", [B_CORE, 64, 64, 64], F32, kind="ExternalOutput")
    # zero-padded 66-pitch staging copy of the input; ExternalOutput =>
    # pre-zeroed by the runtime each invocation, so pad cells are true zeros
    zscr = nc.dram_tensor("zscr", [B_CORE, 4368], F32R, kind="ExternalOutput")

    dbg = {}
    if debug:
        for name, shape in [
            ("dbg_pat", [18, 64, 66]),
            ("dbg_c1in", [128, 34, 34]),
            ("dbg_ein", [128, 18, 18]),
            ("dbg_x", [8, 16, 16]),
            ("dbg_stg", [2, 4, 18, 18]),
            ("dbg_c3in", [128, 17, 2, 17, 2]),
            ("dbg_c4in", [128, 33, 2, 33, 2]),
        ]:
            dbg[name] = nc.dram_tensor(name, shape, F32R, kind="ExternalOutput")

    with tile.TileContext(nc) as tc:
        with (
            tc.tile_pool(name="const", bufs=1) as const,
            tc.tile_pool(name="work", bufs=1) as work,
            tc.tile_pool(name="c1p", bufs=4) as c1p,
            tc.tile_pool(name="einp", bufs=4) as einp,
            tc.tile_pool(name="evp", bufs=4) as evp,
            tc.tile_pool(name="p1p", bufs=3) as p1p,
            tc.tile_pool(name="c3p", bufs=3) as c3p,
            tc.tile_pool(name="c4p", bufs=2) as c4p,
            tc.tile_pool(name="psg", bufs=3, space="PSUM") as psg,
        ):
            # ---------------- identity + relu act-table warm ----------------
            ident = const.tile([128, 128], F32, tag="ident")
            make_identity(nc, ident)
            onesc = const.tile([128, 1], F32R, tag="onesc")
            nc.vector.memset(onesc[:].bitcast(F32), 1.0)
            tblw = const.tile([128, 1], F32, tag="tblw")
            nc.vector.memset(tblw[:], 0.0)
            nc.scalar.activation(tblw[:], tblw[:], AF.Relu, bias=0.0)

            # early weight loads needed for conv0/conv1 builds
            s9 = const.tile([64, 9], F32, tag="w9_c0")
            nc.gpsimd.dma_start(out=s9[:], in_=wc0[:].rearrange("a b c d -> a (b c d)"))
            wsrc1 = const.tile([64, 576], F32, tag="wsrc1")
            nc.gpsimd.dma_start(
                out=wsrc1[:], in_=wc1[:].rearrange("a b c d -> a (b c d)")
            )
            def bias128(dram_b, tag, eng):
                bt = const.tile([128, 1], F32, tag=tag)
                eng.dma_start(
                    out=bt[:],
                    in_=bass.AP(tensor=dram_b[:].tensor, offset=0,
                                ap=[[0, 2], [1, 64], [0, 1]]),
                )
                return bt

            # ---------------- conv0 input staging ---------------------------
            # One padded-copy DMA of the whole input into the pre-zeroed
            # 66-pitch DRAM scratch, then per (pair, sample, dy) tap DMAs:
            # taps t=3*dy+dx of sample s01 sit at partitions 16*s01 + t of a
            # per-pair tile, so pair 0 only waits on its own 6 tap DMAs.
            PP = 64 * 66
            PPb = 64 * 66
            dma_engs = [nc.sync, nc.scalar, nc.gpsimd]
            dma_rr = [0]

            def nxt_eng():
                e = dma_engs[dma_rr[0] % 3]
                dma_rr[0] += 1
                return e

            nc.sync.dma_start(
                out=bass.AP(tensor=zscr[:].tensor, offset=67,
                            ap=[[4368, 8], [66, 64], [1, 64]]),
                in_=bass.AP(tensor=inp[:].tensor, offset=0,
                            ap=[[4096, 8], [66, 64] if False else [64, 64], [1, 64]]),
            )
            pat0 = const.tile([128, 64, 66], F32R, tag="pat0")
            base = pat0[:].offset
            tpat = pat0[:].tensor
            for t, (dy, dx) in enumerate(TAPS):
                nxt_eng().dma_start(
                    out=bass.AP(
                        tensor=tpat,
                        offset=base + t * PPb,
                        ap=[[16 * PPb, 8], [1, PP]],
                    ),
                    in_=bass.AP(
                        tensor=zscr[:].tensor,
                        offset=dy * 66 + dx,
                        ap=[[4368, 8], [1, PP]],
                    ),
                )
            if debug:
                nc.sync.dma_start(out=dbg["dbg_pat"][0:9], in_=pat0[0:9])
                nc.sync.dma_start(out=dbg["dbg_pat"][9:18], in_=pat0[16:25])
            bt0 = bias128(bc0, "bt0", nc.scalar)

            # ---------------- conv0 weights -> [18,128] lhsT ----------------
            ct0w = const.tile([128, 128], F32R, tag="cT_c0")
            nc.vector.memset(ct0w[:].bitcast(F32), 0.0)
            p9 = psg.tile([9, 64], F32, tag="g2")
            nc.tensor.transpose(p9[:], s9[:], ident[0:64, 0:64])
            nc.vector.tensor_scalar_mul(ct0w[0:9, 0:64], p9[:], 1.0)
            nc.sync.dma_start(out=ct0w[16:25, 64:128], in_=ct0w[0:9, 0:64])
            for p, eng in ((1, nc.scalar), (2, nc.gpsimd), (3, nc.sync)):
                eng.dma_start(
                    out=ct0w[32 * p : 32 * p + 25, :], in_=ct0w[0:25, :]
                )
            wsrc3 = const.tile([64, 576], F32, tag="wsrc3")
            nc.sync.dma_start(
                out=wsrc3[:], in_=wc3[:].rearrange("a b c d -> a (b c d)")
            )
            wsrc4 = const.tile([64, 576], F32, tag="wsrc4")
            nc.sync.dma_start(
                out=wsrc4[:], in_=wc4[:].rearrange("a b c d -> a (b c d)")
            )

            # ================ conv0 + pool (4 groups of 2 banks per pair) ===
            c1in_l = []
            for p in range(NPAIR):
                c1in = c1p.tile([128, 34, 34], F32R, tag="c1in")
                nc.vector.memset(c1in[:, 0:1, :].bitcast(F32), 0.0)
                nc.vector.memset(c1in[:, 33:34, :].bitcast(F32), 0.0)
                nc.vector.memset(c1in[:, 1:33, 0:1].bitcast(F32), 0.0)
                nc.vector.memset(c1in[:, 1:33, 33:34].bitcast(F32), 0.0)
                for g in range(4):
                    ps = psg.tile([128, 2, 4, 2, 32, 2], F32, tag="g2")
                    for n2 in range(2):
                        n = 2 * g + n2
                        nc.tensor.matmul(
                            ps[:, n2],
                            ct0w[32 * p : 32 * p + 25, :],
                            pat0[32 * p : 32 * p + 25, n * 8 : n * 8 + 8, 0:64],
                            start=True,
                            stop=True,
                            tile_position=(32 * p, 0),
                        )
                    R = evp.tile([128, 2, 4, 2, 32, 2], F32, tag="evict")
                    if g % 2 == 0:
                        nc.scalar.activation(R[:], ps[:], AF.Relu, bias=bt0)
                    else:
                        nc.vector.tensor_scalar(
                            R[:], ps[:], bt0[:], 0.0, ALU.add, ALU.max
                        )
                    T1 = p1p.tile([128, 2, 4, 2, 32], F32, tag="pool1")
                    nc.vector.tensor_add(
                        T1[:], R[:, :, :, :, :, 0], R[:, :, :, :, :, 1]
                    )
                    nc.gpsimd.tensor_add(
                        c1in[:, 1 + 8 * g : 9 + 8 * g, 1:33],
                        T1[:, :, :, 0, :].rearrange("p a b c -> p (a b) c"),
                        T1[:, :, :, 1, :].rearrange("p a b c -> p (a b) c"),
                    )
                c1in_l.append(c1in)

            wencs = const.tile([64, 9], F32, tag="wencs")
            nc.sync.dma_start(
                out=wencs[:], in_=wen[:].rearrange("a b c d -> (a b) (c d)")
            )
            encT = const.tile([128, 9, 2], F32R, tag="encT")
            nc.vector.memset(encT[:].bitcast(F32), 0.0)
            for t in range(9):
                nc.scalar.activation(
                    encT[0:64, t, 0:1], wencs[:, t : t + 1], AF.Copy,
                    bias=0.0, scale=0.25,
                )
            nc.gpsimd.dma_start(out=encT[64:128, :, 1:2], in_=encT[0:64, :, 0:1])
            bte = const.tile([2, 1], F32, tag="bte")
            nc.sync.dma_start(
                out=bte[:],
                in_=bass.AP(tensor=ben[:].tensor, offset=0, ap=[[0, 2], [1, 1]]),
            )

            # ---------------- conv1 weights (block-diag taps) ---------------
            def build_wtap(tag, wsrc_tile, scale):
                wt = const.tile([128, 9, 128], F32R, tag=tag)
                nc.vector.memset(wt[:].bitcast(F32), 0.0)
                for t in range(9):
                    pw = psg.tile([64, 64], F32, tag="g2")
                    nc.tensor.transpose(pw[:], wsrc_tile[:, t::9], ident[0:64, 0:64])
                    if scale == 1.0:
                        nc.vector.tensor_scalar_mul(wt[0:64, t, 0:64], pw[:], 1.0)
                    else:
                        nc.scalar.activation(
                            wt[0:64, t, 0:64], pw[:], AF.Copy, bias=0.0, scale=scale
                        )
                nc.sync.dma_start(out=wt[64:128, :, 64:128], in_=wt[0:64, :, 0:64])
                return wt

            wtap1 = build_wtap("wtap_c1", wsrc1, 0.25)  # avg-pool folded in
            bt1 = bias128(bc1, "bt1", nc.sync)

            # ================ conv1 + pool (one 2-bank group per pair) ======
            ein_l = []
            for p in range(NPAIR):
                c1in = c1in_l[p]
                e_in = einp.tile([128, 18, 18], F32R, tag="e_in")
                nc.vector.memset(e_in[:, 0:1, :].bitcast(F32), 0.0)
                nc.vector.memset(e_in[:, 17:18, :].bitcast(F32), 0.0)
                nc.vector.memset(e_in[:, 1:17, 0:1].bitcast(F32), 0.0)
                nc.vector.memset(e_in[:, 1:17, 17:18].bitcast(F32), 0.0)
                ps = psg.tile([128, 2, 8, 2, 16, 2], F32, tag="g2")
                for n in range(2):
                    for t, (dy, dx) in enumerate(TAPS):
                        nc.tensor.matmul(
                            ps[:, n],
                            wtap1[:, t, :],
                            c1in[:, n * 16 + dy : n * 16 + dy + 16, dx : dx + 32],
                            start=(t == 0),
                            stop=(t == 8),
                        )
                R = evp.tile([128, 2, 8, 2, 16, 2], F32, tag="evict")
                if p % 2 == 0:
                    nc.scalar.activation(R[:], ps[:], AF.Relu, bias=bt1)
                else:
                    nc.vector.tensor_scalar(R[:], ps[:], bt1[:], 0.0, ALU.add, ALU.max)
                T1 = p1p.tile([128, 2, 8, 2, 16], F32, tag="pool1")
                nc.vector.tensor_add(T1[:], R[:, :, :, :, :, 0], R[:, :, :, :, :, 1])
                nc.gpsimd.tensor_add(
                    e_in[:, 1:17, 1:17],
                    T1[:, :, :, 0, :].rearrange("p a b c -> p (a b) c"),
                    T1[:, :, :, 1, :].rearrange("p a b c -> p (a b) c"),
                )
                if debug and p == 0:
                    nc.sync.dma_start(out=dbg["dbg_ein"][:], in_=e_in[:])
                ein_l.append(e_in)

            # sigmoid/tanh table load overlaps late encoder
            nc.scalar.activation(tblw[:], tblw[:], AF.Sigmoid, bias=0.0)

            # ---------------- deferred weights: enc, conv2, conv3, NTM ------

            s9c2 = const.tile([64, 9], F32R, tag="w9_c2")
            nc.gpsimd.dma_start(
                out=s9c2[:], in_=wc2[:].rearrange("a b c d -> a (b c d)")
            )
            c2T = const.tile([2, 128, 9], F32R, tag="c2T")
            nc.vector.memset(c2T[:].bitcast(F32), 0.0)
            for r in range(2):
                nc.gpsimd.dma_start(
                    out=bass.AP(
                        tensor=c2T[:].tensor,
                        offset=c2T[:].offset + r * (128 * 9) + r * 576,
                        ap=[[128 * 9, 1], [1, 576]],
                    ),
                    in_=bass.AP(
                        tensor=s9c2[:].tensor, offset=s9c2[:].offset,
                        ap=[[9, 64], [1, 9]],
                    ),
                )
            stg2 = const.tile([2, 4, 18, 18], F32R, tag="stg2")
            nc.vector.memset(stg2[:].bitcast(F32), 0.0)

            wx = const.tile([128, 2, 768], F32R, tag="wx")
            for kt in range(2):
                nc.sync.dma_start(
                    out=wx[:, kt, 0:256],
                    in_=wlx[kt * 128 : (kt + 1) * 128, 0:256],
                )
                nc.sync.dma_start(
                    out=wx[:, kt, 256:768],
                    in_=wlx[kt * 128 : (kt + 1) * 128, 512:1024],
                )
            bigo = const.tile([128, 6], F32, tag="bigo")
            nc.sync.dma_start(
                out=bigo[:, 0:2],
                in_=bass.AP(tensor=bls[:].tensor, offset=0,
                            ap=[[1, 128], [128, 2]]),
            )
            nc.sync.dma_start(
                out=bigo[:, 2:6],
                in_=bass.AP(tensor=bls[:].tensor, offset=512,
                            ap=[[1, 128], [128, 4]]),
            )
            wp3 = const.tile([128, 2, 3], F32R, tag="wp3")
            for kt in range(2):
                nc.sync.dma_start(
                    out=wp3[:, kt, :],
                    in_=bass.AP(
                        tensor=wpa[:].tensor,
                        offset=kt * 128 * 3108 + 261,
                        ap=[[3108, 128], [262, 3]],
                    ),
                )
            bp3 = const.tile([3, 1], F32, tag="bp3")
            nc.sync.dma_start(
                out=bp3[:],
                in_=bass.AP(tensor=bpa[:].tensor, offset=261, ap=[[262, 3], [1, 1]]),
            )
            wo = const.tile([128, 2, 256], F32R, tag="wo")
            for kt in range(2):
                nc.sync.dma_start(
                    out=wo[:, kt, :], in_=wou[kt * 128 : (kt + 1) * 128, :]
                )
            w2c = const.tile([128, 6, 256], F32R, tag="w2c")
            nc.gpsimd.dma_start(
                out=w2c[:],
                in_=bass.AP(tensor=wou[:].tensor, offset=256 * 256,
                            ap=[[256, 128], [128 * 256, 6], [1, 256]]),
            )
            ones3 = const.tile([128, 6, 3], F32R, tag="ones3")
            nc.vector.memset(ones3[:].bitcast(F32), 0.0)
            for c in range(6):
                nc.vector.memset(ones3[:, c, c // 2 : c // 2 + 1].bitcast(F32), 1.0)
            rhs2 = const.tile([4, 256], F32R, tag="rhs2")
            nc.sync.dma_start(out=rhs2[3:4, :], in_=bou[:].unsqueeze(0))
            lhsT2 = const.tile([4, 8], F32R, tag="lhsT2")
            nc.vector.memset(lhsT2[:].bitcast(F32), 1.0)
            bt2 = bias128(bc2, "bt2", nc.sync)
            bt3 = bias128(bc3, "bt3", nc.sync)
            bt4 = bias128(bc4, "bt4", nc.sync)

            wtap3 = build_wtap("wtap_c3", wsrc3, 1.0)

            # ================ enc conv (M=2 per pair, DMA into xstage) ======
            xstage = const.tile([8, 16, 16], F32, tag="xstage")
            scrw = psg.tile([1, 512], F32, tag="ntm", bufs=1, name="scrw")
            wdst = scrw[:]

            def warm(dep, n=1):
                ap = dep
                free = 1
                for dd in ap.ap[1:]:
                    free *= dd[1]
                reps = 512 // free
                k = ap.ap[0][1]
                rhs = bass.AP(
                    tensor=ap.tensor, offset=ap.offset,
                    ap=[list(ap.ap[0])] + [[0, reps]] + [list(dd) for dd in ap.ap[1:]],
                )
                for _ in range(n):
                    nc.tensor.matmul(wdst, onesc[0:k, 0:1], rhs, start=True, stop=True)

            estage_l = []
            for p in range(NPAIR):
                e_in = ein_l[p]
                pe = psg.tile([2, 16, 16], F32, tag="g2")
                for t, (dy, dx) in enumerate(TAPS):
                    nc.tensor.matmul(
                        pe[:],
                        encT[:, t, :],
                        e_in[:, dy : dy + 16, dx : dx + 16],
                        start=(t == 0),
                        stop=(t == 8),
                    )
                estage = work.tile([2, 16, 16], F32R, tag=f"estage{p}")
                nc.scalar.activation(estage[:], pe[:], AF.Relu, bias=bte)
                nc.gpsimd.dma_start(out=xstage[2 * p : 2 * p + 2, :, :], in_=estage[:])
                warm(estage[:], n=2)
                estage_l.append(estage)

            if debug:
                nc.gpsimd.dma_start(out=dbg["dbg_x"][:], in_=xstage[:])
            # ================ NTM step ======================================
            xT = work.tile([128, 2, 8], F32R, tag="xT")
            for kt in range(2):
                pxt = psg.tile([128, 8], F32, tag="g2")
                nc.tensor.transpose(
                    pxt[:],
                    xstage[:].rearrange("p a b -> p (a b)")[:, kt * 128 : kt * 128 + 128],
                    ident[0:8, 0:8],
                )
                nc.scalar.activation(xT[:, kt, :], pxt[:], AF.Copy, bias=0.0, scale=1.0)

            # w_out reads-part colsums -> rhs2 rows 0:3
            pcs = psg.tile([3, 256], F32, tag="g2")
            for c in range(6):
                nc.tensor.matmul(
                    pcs[:], ones3[:, c, :], w2c[:, c, :],
                    start=(c == 0), stop=(c == 5),
                )
            nc.scalar.activation(rhs2[0:3, :], pcs[:], AF.Copy, bias=0.0, scale=1.0)

            # z = x @ Wx + b for gates i, g, o
            zps = psg.tile([128, 6, 8], F32, tag="g2")
            for j in range(3):
                for h2 in range(2):
                    for kt in range(2):
                        nc.tensor.matmul(
                            zps[:, 2 * j + h2, :],
                            wx[:, kt, j * 256 + h2 * 128 : j * 256 + h2 * 128 + 128],
                            xT[:, kt, :],
                            start=(kt == 0),
                            stop=(kt == 1),
                        )
            zb = work.tile([128, 6, 8], F32, tag="zb")
            bigo_b = bass.AP(
                tensor=bigo[:].tensor, offset=bigo[:].offset,
                ap=[list(d) for d in bigo[:].ap] + [[0, 8]],
            )
            nc.vector.tensor_tensor(zb[:], zps[:], bigo_b, op=ALU.add)

            si = work.tile([128, 2, 8], F32R, tag="gate0")
            nc.scalar.activation(si[:], zb[:, 0:2, :], AF.Sigmoid, bias=0.0)
            tg = work.tile([128, 2, 8], F32R, tag="gate1")
            nc.scalar.activation(tg[:], zb[:, 2:4, :], AF.Tanh, bias=0.0)
            so = work.tile([128, 2, 8], F32R, tag="gate2")
            nc.scalar.activation(so[:], zb[:, 4:6, :], AF.Sigmoid, bias=0.0)
            warm(si[:], n=3)
            ctile = work.tile([128, 2, 8], F32R, tag="ctile")
            nc.vector.tensor_mul(ctile[:], si[:], tg[:])
            warm(so[:], n=3)
            tct = work.tile([128, 2, 8], F32R, tag="tct")
            nc.scalar.activation(tct[:], ctile[:], AF.Tanh, bias=0.0)
            warm(ctile[:], n=3)
            h = work.tile([128, 2, 8], F32R, tag="h")
            nc.vector.tensor_mul(h[:], so[:], tct[:])
            warm(tct[:], n=3)

            # conv4 taps built inside the NTM window (PE work + DVE evicts)
            wt4 = const.tile([128, 9, 128], F32R, tag="wtap_c4")
            nc.gpsimd.memset(wt4[:].bitcast(F32), 0.0)

            pp3 = psg.tile([3, 8], F32, tag="g2")
            for kt in range(2):
                nc.tensor.matmul(
                    pp3[:], wp3[:, kt, :], h[:, kt, :], start=(kt == 0), stop=(kt == 1)
                )
            t1 = work.tile([3, 8], F32R, tag="t1")
            nc.scalar.activation(t1[:], pp3[:], AF.Identity, bias=bp3)
            t2 = work.tile([3, 8], F32R, tag="t2")
            nc.vector.tensor_scalar(t2[:], t1[:], -CLIP, CLIP, ALU.max, ALU.min)
            for t in range(4):
                pw = psg.tile([64, 64], F32, tag="g2")
                nc.tensor.transpose(pw[:], wsrc4[:, t::9], ident[0:64, 0:64])
                nc.vector.tensor_scalar_mul(wt4[0:64, t, 0:64], pw[:], 1.0)
            eu = work.tile([3, 8], F32R, tag="eu")
            nc.scalar.activation(eu[:], t2[:], AF.Exp, bias=0.0)
            ev = work.tile([3, 8], F32R, tag="ev")
            nc.vector.tensor_scalar_add(ev[:], eu[:], 1.0)
            warm(t2[:], n=3)
            sp = work.tile([3, 8], F32R, tag="sp")
            nc.scalar.activation(sp[:], ev[:], AF.Ln, bias=0.0)
            for t in range(4, 9):
                pw = psg.tile([64, 64], F32, tag="g2")
                nc.tensor.transpose(pw[:], wsrc4[:, t::9], ident[0:64, 0:64])
                nc.vector.tensor_scalar_mul(wt4[0:64, t, 0:64], pw[:], 1.0)
            q = work.tile([3, 8], F32R, tag="q")
            nc.scalar.activation(q[:], sp[:], AF.Exp, bias=0.0, scale=-LN64)
            qe = work.tile([3, 8], F32R, tag="qe")
            nc.vector.tensor_scalar_add(qe[:], q[:], 1e-8)
            warm(sp[:], n=3)
            rec = work.tile([3, 8], F32, tag="rec")
            nc.vector.reciprocal(rec[:], qe[:])
            nc.vector.scalar_tensor_tensor(
                out=lhsT2[0:3, :], in0=q[:], scalar=1e-6, in1=rec[:],
                op0=ALU.mult, op1=ALU.mult,
            )
            nc.gpsimd.dma_start(out=wt4[64:128, :, 64:128], in_=wt4[0:64, :, 0:64])
            warm(qe[:], n=3)

            # out = clip(h @ w_out[:256] + reads @ w_out[256:] + b_out),
            # per pair (M=2) so the clip lands lane-aligned in stg2
            pout = psg.tile([2, 4, 16, 16], F32, tag="g2")
            for p in range(NPAIR):
                for kt in range(2):
                    nc.tensor.matmul(
                        pout[:, p],
                        h[:, kt, 2 * p : 2 * p + 2],
                        wo[:, kt, :],
                        start=(kt == 0),
                        stop=False,
                    )
                nc.tensor.matmul(
                    pout[:, p],
                    lhsT2[:, 2 * p : 2 * p + 2],
                    rhs2[:],
                    start=False,
                    stop=True,
                )
                nc.vector.tensor_scalar(
                    stg2[:, p, 1:17, 1:17], pout[:, p],
                    -CLIP, CLIP, ALU.max, ALU.min,
                )

            if debug:
                nc.sync.dma_start(out=dbg["dbg_stg"][:], in_=stg2[:])
            # ================ decoder ======================================
            c3in_l = []
            ps2_l = []

            def conv2_pair(p):
                ps2 = psg.tile([128, 16, 16], F32, tag="g2")
                for t, (dy, dx) in enumerate(TAPS):
                    nc.tensor.matmul(
                        ps2[:],
                        c2T[:, :, t],
                        stg2[:, p, dy : dy + 16, dx : dx + 16],
                        start=(t == 0),
                        stop=(t == 8),
                    )
                ps2_l.append(ps2)

            def c3asm_pair(p):
                psv = ps2_l[p][:]
                c3in = c3p.tile([128, 17, 2, 17, 2], F32R, tag="c3in")
                nc.gpsimd.memset(c3in[:, 0, 0, :, :].bitcast(F32), 0.0)
                nc.gpsimd.memset(c3in[:, 16, 1, :, :].bitcast(F32), 0.0)
                nc.gpsimd.memset(c3in[:, :, :, 0, 0].bitcast(F32), 0.0)
                nc.gpsimd.memset(c3in[:, :, :, 16, 1].bitcast(F32), 0.0)
                nc.scalar.activation(c3in[:, 0:16, 1, 0:16, 1], psv, AF.Relu, bias=bt2)
                nc.vector.tensor_scalar(
                    c3in[:, 0:16, 1, 1:17, 0], psv, bt2[:], 0.0, ALU.add, ALU.max
                )
                nc.scalar.activation(c3in[:, 1:17, 0, 0:16, 1], psv, AF.Relu, bias=bt2)
                nc.vector.tensor_scalar(
                    c3in[:, 1:17, 0, 1:17, 0], psv, bt2[:], 0.0, ALU.add, ALU.max
                )
                if debug and p == 0:
                    nc.sync.dma_start(out=dbg["dbg_c3in"][:], in_=c3in[:])
                c3in_l.append(c3in)

            c4in_l = []

            def conv3_pair(p):
                c3v = c3in_l[p][:].rearrange("p r a c b -> p (r a) (c b)")
                ps = psg.tile([128, 2, 16, 32], F32, tag="g2")
                for n in range(2):
                    for t, (dy, dx) in enumerate(TAPS):
                        nc.tensor.matmul(
                            ps[:, n],
                            wtap3[:, t, :],
                            c3v[:, n * 16 + dy : n * 16 + dy + 16, dx : dx + 32],
                            start=(t == 0),
                            stop=(t == 8),
                        )
                psv = ps[:].rearrange("p n a c -> p (n a) c")
                c4in = c4p.tile([128, 33, 2, 33, 2], F32R, tag="c4in")
                nc.gpsimd.memset(c4in[:, 0, 0, :, :].bitcast(F32), 0.0)
                nc.gpsimd.memset(c4in[:, 32, 1, :, :].bitcast(F32), 0.0)
                nc.gpsimd.memset(c4in[:, :, :, 0, 0].bitcast(F32), 0.0)
                nc.gpsimd.memset(c4in[:, :, :, 32, 1].bitcast(F32), 0.0)
                nc.scalar.activation(c4in[:, 0:32, 1, 0:32, 1], psv, AF.Relu, bias=bt3)
                nc.vector.tensor_scalar(
                    c4in[:, 0:32, 1, 1:33, 0], psv, bt3[:], 0.0, ALU.add, ALU.max
                )
                nc.scalar.activation(c4in[:, 1:33, 0, 0:32, 1], psv, AF.Relu, bias=bt3)
                nc.vector.tensor_scalar(
                    c4in[:, 1:33, 0, 1:33, 0], psv, bt3[:], 0.0, ALU.add, ALU.max
                )
                if debug and p == 0:
                    nc.sync.dma_start(out=dbg["dbg_c4in"][:], in_=c4in[:])
                c4in_l.append(c4in)

            def conv4_pair(p):
                c4v = c4in_l[p][:].rearrange("p r a c b -> p (r a) (c b)")
                for g in range(4):
                    ps = psg.tile([128, 2, 8, 64], F32, tag="g2")
                    for t, (dy, dx) in enumerate(TAPS):
                        for n2 in range(2):
                            n = 2 * g + n2
                            nc.tensor.matmul(
                                ps[:, n2],
                                wt4[:, t, :],
                                c4v[:, n * 8 + dy : n * 8 + dy + 8, dx : dx + 64],
                                start=(t == 0),
                                stop=(t == 8),
                            )
                    R = evp.tile([128, 2, 8, 64], F32, tag="evict")
                    if g % 2 == 0:
                        nc.scalar.activation(R[:], ps[:], AF.Relu, bias=bt4)
                    else:
                        nc.vector.tensor_scalar(
                            R[:], ps[:], bt4[:], 0.0, ALU.add, ALU.max
                        )
                    for s01 in range(2):
                        eng = (nc.sync, nc.gpsimd)[s01]
                        eng.dma_start(
                            out=out[2 * p + s01, :, 16 * g : 16 * g + 16, :],
                            in_=R[64 * s01 : 64 * s01 + 64].rearrange(
                                "p n a c -> p (n a) c"
                            ),
                        )

            conv2_pair(0)
            c3asm_pair(0)
            conv2_pair(1)
            c3asm_pair(1)
            conv2_pair(2)
            c3asm_pair(2)
            conv3_pair(0)
            conv2_pair(3)
            c3asm_pair(3)
            conv3_pair(1)
            conv4_pair(0)
            conv3_pair(2)
            conv4_pair(1)
            conv3_pair(3)
            conv4_pair(2)
            conv4_pair(3)

    nc.compile()
    return nc


_NC_CACHE = {}
LAST_RESULT = None

WEIGHT_NAMES = [
    "w_conv0", "b_conv0", "w_conv1", "b_conv1", "w_enc", "b_enc",
    "w_conv2", "b_conv2", "w_conv3", "b_conv3", "w_conv4", "b_conv4",
    "w_lstm_x", "b_lstm", "w_param", "b_param", "w_out", "b_out",
]


def kernel(**inputs):
    global LAST_RESULT
    from concourse.bass_utils import run_bass_kernel_spmd

    debug = bool(int(os.environ.get("KDEBUG", "0")))
    key = ("nc", debug)
    if key not in _NC_CACHE:
        _NC_CACHE[key] = build_nc(debug=debug)
    nc = _NC_CACHE[key]

    xs = np.ascontiguousarray(np.asarray(inputs["inputs"], dtype=np.float32))
    weights = {
        k: np.ascontiguousarray(np.asarray(inputs[k], dtype=np.float32))
        for k in WEIGHT_NAMES
    }
    in_maps = []
    for c in range(N_CORES):
        m = dict(weights)
        m["inputs"] = xs[c * B_CORE : (c + 1) * B_CORE]
        in_maps.append(m)

    res = run_bass_kernel_spmd(nc, in_maps, core_ids=list(range(N_CORES)))
    LAST_RESULT = res
    return np.concatenate([r["out"] for r in res.results], axis=0)


if __name__ == "__main__":
    nc = build_nc()
    print("built ok")
